# revision 2
# baseline (speedup 1.0000x reference)
"""GQA multi-head attention kernel for 8 Trainium2 NeuronCores.

Problem: B=4, T=2048, C=1024, G=4 KV groups x 4 query heads/group, D=64.
Sharding: core = (batch b, group-half gh). Each core handles one batch and
two KV groups (8 query heads), computes a partial output projection over
its 512 head-dims; host sums the two partials per batch and adds bias.
"""

import numpy as np
import ml_dtypes

import concourse.bass as bass
import concourse.mybir as mybir
import concourse.tile as tile
from concourse import bacc
from concourse._compat import with_exitstack
from concourse.bass_utils import run_bass_kernel_spmd

P = 128
B, T, C = 4, 2048, 1024
G, HPG, D = 4, 4, 64
NPAIR = 4          # head-pairs per core (head j of group A + head j of group B)
CT = C // P        # 8 contraction tiles
ST = T // P        # 16 s-tiles
W = 512            # tq chunk width / matmul free dim
NC_CHUNKS = T // W # 4 tq chunks

F32 = mybir.dt.float32
BF16 = mybir.dt.bfloat16
AF = mybir.ActivationFunctionType
ALU = mybir.AluOpType


@with_exitstack
def _attention_kernel(ctx, tc):
    nc = tc.nc
    xT = nc.dram_tensor("xT", [C, T], F32, kind="ExternalInput")
    wq = nc.dram_tensor("wq", [C, NPAIR * P], F32, kind="ExternalInput")
    wk = nc.dram_tensor("wk", [C, P], F32, kind="ExternalInput")
    wv = nc.dram_tensor("wv", [C, P], F32, kind="ExternalInput")
    wp = nc.dram_tensor("wp", [NPAIR * P, C], F32, kind="ExternalInput")
    tri = nc.dram_tensor("tri", [P, P], BF16, kind="ExternalInput")
    out = nc.dram_tensor("out", [T, C], F32, kind="ExternalOutput")

    stage = ctx.enter_context(tc.tile_pool(name="stage", bufs=1))
    persist = ctx.enter_context(tc.tile_pool(name="persist", bufs=1))
    work = ctx.enter_context(tc.tile_pool(name="work", bufs=3))
    ocpool = ctx.enter_context(tc.tile_pool(name="ocpool", bufs=2))
    psum2 = ctx.enter_context(tc.tile_pool(name="psum2", bufs=2, space="PSUM"))
    psum1 = ctx.enter_context(tc.tile_pool(name="psum1", bufs=1, space="PSUM"))
    psumm = ctx.enter_context(tc.tile_pool(name="psumm", bufs=1, space="PSUM"))

    # ---- load inputs ----
    xT_sb = stage.tile([P, CT, T], F32)
    xT_r = xT.rearrange("(o p) t -> p o t", p=P)
    for ct in range(CT):
        nc.sync.dma_start(xT_sb[:, ct, :], xT_r[:, ct, :])
    wq_sb = stage.tile([P, CT, NPAIR * P], F32)
    nc.sync.dma_start(wq_sb[:], wq.rearrange("(o p) m -> p o m", p=P))
    wk_sb = stage.tile([P, CT, P], F32)
    nc.sync.dma_start(wk_sb[:], wk.rearrange("(o p) m -> p o m", p=P))
    wv_sb = stage.tile([P, CT, P], F32)
    nc.sync.dma_start(wv_sb[:], wv.rearrange("(o p) m -> p o m", p=P))
    wp_sb = persist.tile([P, NPAIR, C], F32)
    nc.sync.dma_start(wp_sb[:], wp.rearrange("(o p) c -> p o c", p=P))
    tri_sb = persist.tile([P, P], BF16)
    nc.sync.dma_start(tri_sb[:], tri[:])

    ones_sb = persist.tile([65, 64], F32)
    nc.vector.memset(ones_sb[:], 1.0)

    qT = persist.tile([P, NPAIR, T], F32)   # [d(2 heads), pair, t]
    kT = persist.tile([P, T], F32)          # [d(2 groups), s]
    v_sb = persist.tile([P, ST, 130], BF16) # [s, s-tile, V_A|1|V_B|1]
    nc.vector.memset(v_sb[:, :, 64:65], 1.0)
    nc.vector.memset(v_sb[:, :, 129:130], 1.0)

    # ---- K^T projection: kT = wk.T @ xT ----
    for c4 in range(NC_CHUNKS):
        ps = psum2.tile([P, W], F32, tag="sA")
        for ct in range(CT):
            nc.tensor.matmul(
                ps, wk_sb[:, ct, :], xT_sb[:, ct, c4 * W:(c4 + 1) * W],
                start=(ct == 0), stop=(ct == CT - 1))
        nc.vector.tensor_copy(kT[:, c4 * W:(c4 + 1) * W], ps)

    # ---- V projection: v = xT.T @ wv (t on partitions) ----
    for i in range(ST):
        ps = psum2.tile([P, P], F32, tag="sB")
        for ct in range(CT):
            nc.tensor.matmul(
                ps, xT_sb[:, ct, i * P:(i + 1) * P], wv_sb[:, ct, :],
                start=(ct == 0), stop=(ct == CT - 1))
        nc.vector.tensor_copy(v_sb[:, i, 0:64], ps[:, 0:64])
        nc.vector.tensor_copy(v_sb[:, i, 65:129], ps[:, 64:128])

    # ---- Q^T projection: qT[:, j] = wq[:, j].T @ xT ----
    for j in range(NPAIR):
        for c4 in range(NC_CHUNKS):
            ps = psum2.tile([P, W], F32, tag="sA")
            for ct in range(CT):
                nc.tensor.matmul(
                    ps, wq_sb[:, ct, j * P:(j + 1) * P],
                    xT_sb[:, ct, c4 * W:(c4 + 1) * W],
                    start=(ct == 0), stop=(ct == CT - 1))
            nc.vector.tensor_copy(qT[:, j, c4 * W:(c4 + 1) * W], ps)

    # ---- attention + projection, per tq chunk ----
    out_r = out.rearrange("(tb p) c -> tb p c", p=P)
    for c in range(NC_CHUNKS):
        oc = ocpool.tile([P, NPAIR, W], F32, tag="oc")  # normalized O^T chunk
        for j in range(NPAIR):
            oA = psum1.tile([P, W], F32, tag="oA")
            oB = psum1.tile([P, W], F32, tag="oB")
            nst = 4 * c + 4
            for i in range(nst):
                o = max(0, (i - 4 * c) * P)
                w = W - o
                psA = psum2.tile([P, W], F32, tag="sA")
                psB = psum2.tile([P, W], F32, tag="sB")
                # S^T = K @ Q^T for both groups, packed into PE row halves
                nc.tensor.matmul(
                    psA[:, 0:w], kT[0:64, i * P:(i + 1) * P],
                    qT[0:64, j, c * W + o:(c + 1) * W], start=True, stop=True)
                nc.tensor.matmul(
                    psB[:, 0:w], kT[64:128, i * P:(i + 1) * P],
                    qT[64:128, j, c * W + o:(c + 1) * W], start=True, stop=True)
                pA = work.tile([P, W], BF16, tag="pA")
                pB = work.tile([P, W], BF16, tag="pB")
                nc.scalar.activation(pA[:, 0:w], psA[:, 0:w], AF.Exp, scale=0.125)
                nc.scalar.activation(pB[:, 0:w], psB[:, 0:w], AF.Exp, scale=0.125)
                if i >= 4 * c:  # diagonal s-tile: causal mask
                    nc.vector.tensor_tensor(pA[:, 0:P], pA[:, 0:P], tri_sb[:], ALU.mult)
                    nc.vector.tensor_tensor(pB[:, 0:P], pB[:, 0:P], tri_sb[:], ALU.mult)
                # O^T[0:64] += V.T @ P^T ; row 64 accumulates the softmax denom
                nc.tensor.matmul(
                    oA[0:65, o:W], v_sb[:, i, 0:65], pA[:, 0:w],
                    start=(i == 0), stop=(i == nst - 1), skip_group_check=True)
                nc.tensor.matmul(
                    oB[0:65, o:W], v_sb[:, i, 65:130], pB[:, 0:w],
                    start=(i == 0), stop=(i == nst - 1), skip_group_check=True)
            # normalize: O^T[d, tq] / denom[tq]
            rA = work.tile([65, W], F32, tag="rA")
            rB = work.tile([65, W], F32, tag="rB")
            nc.vector.reciprocal(rA[64:65, :], oA[64:65, :])
            nc.vector.reciprocal(rB[64:65, :], oB[64:65, :])
            bcA = psumm.tile([64, W], F32, tag="bc")
            bcB = psumm.tile([64, W], F32, tag="bc")
            nc.tensor.matmul(bcA, ones_sb[64:65, :], rA[64:65, :], start=True, stop=True)
            nc.tensor.matmul(bcB, ones_sb[64:65, :], rB[64:65, :], start=True, stop=True)
            bcsA = work.tile([64, W], F32, tag="bcsA")
            bcsB = work.tile([64, W], F32, tag="bcsB")
            nc.vector.tensor_copy(bcsA, bcA)
            nc.vector.tensor_copy(bcsB, bcB)
            nc.vector.tensor_tensor(oc[0:64, j, :], oA[0:64, :], bcsA[:], ALU.mult)
            stb = work.tile([64, W], F32, tag="stb")
            nc.vector.tensor_tensor(stb[:], oB[0:64, :], bcsB[:], ALU.mult)
            nc.sync.dma_start(oc[64:128, j, :], stb[:])  # shift to partitions 64:128
        # ---- output projection for this chunk's 4 t-blocks ----
        for k in range(4):
            tb = 4 * c + k
            for cb in range(2):
                po = psumm.tile([P, W], F32, tag="po")
                for j in range(NPAIR):
                    nc.tensor.matmul(
                        po, oc[:, j, k * P:(k + 1) * P],
                        wp_sb[:, j, cb * W:(cb + 1) * W],
                        start=(j == 0), stop=(j == NPAIR - 1))
                os_t = work.tile([P, W], F32, tag="os")
                nc.vector.tensor_copy(os_t, po)
                nc.sync.dma_start(out_r[tb, :, cb * W:(cb + 1) * W], os_t)


_NC_CACHE = None


def build_nc():
    global _NC_CACHE
    if _NC_CACHE is None:
        nc = bacc.Bacc(name="gqa_attn")
        with tile.TileContext(nc) as tc:
            _attention_kernel(tc)
        nc.finalize()
        _NC_CACHE = nc
    return _NC_CACHE


def make_core_inputs(x, Wq, Wk, Wv, Wp):
    """Build the 8 per-core input dicts from full fp32 inputs."""
    x = np.asarray(x, dtype=np.float32)
    Wq = np.asarray(Wq, dtype=np.float32)
    Wk = np.asarray(Wk, dtype=np.float32)
    Wv = np.asarray(Wv, dtype=np.float32)
    Wp = np.asarray(Wp, dtype=np.float32)
    tri = np.triu(np.ones((P, P), dtype=np.float32)).astype(ml_dtypes.bfloat16)
    in_maps = []
    for b in range(B):
        xTb = np.ascontiguousarray(x[b].T)  # [C, T]
        for gh in range(2):
            ga, gb = 2 * gh, 2 * gh + 1
            # wq: pair-block j = [Wq[ga, j] | Wq[gb, j]]  -> [C, 512]
            wq_c = np.concatenate(
                [np.concatenate([Wq[ga, j], Wq[gb, j]], axis=1) for j in range(HPG)],
                axis=1)
            wk_c = np.concatenate([Wk[ga], Wk[gb]], axis=1)  # [C, 128]
            wv_c = np.concatenate([Wv[ga], Wv[gb]], axis=1)  # [C, 128]
            # wp rows: k-tile j = [rows of head (ga, j); rows of head (gb, j)]
            wp_rows = []
            for j in range(HPG):
                wp_rows.append(Wp[(ga * HPG + j) * D:(ga * HPG + j + 1) * D])
                wp_rows.append(Wp[(gb * HPG + j) * D:(gb * HPG + j + 1) * D])
            wp_c = np.concatenate(wp_rows, axis=0)  # [512, C]
            in_maps.append({
                "xT": np.ascontiguousarray(xTb),
                "wq": np.ascontiguousarray(wq_c),
                "wk": np.ascontiguousarray(wk_c),
                "wv": np.ascontiguousarray(wv_c),
                "wp": np.ascontiguousarray(wp_c),
                "tri": tri,
            })
    return in_maps


def combine_outputs(results, bp):
    bp = np.asarray(bp, dtype=np.float32)
    outs = []
    for b in range(B):
        outs.append(results[2 * b]["out"] + results[2 * b + 1]["out"] + bp)
    return np.stack(outs, axis=0)


def kernel(x, Wq, Wk, Wv, Wp, bp):
    nc = build_nc()
    in_maps = make_core_inputs(x, Wq, Wk, Wv, Wp)
    res = run_bass_kernel_spmd(nc, in_maps, core_ids=list(range(8)))
    return combine_outputs(res.results, bp)
